# revision 1
# baseline (speedup 1.0000x reference)
"""Multi-head attention (B=4, Q=K=2048, D=512, H=8) on 8 TRN2 NeuronCores.

Sharding: data-parallel over batch across core pairs (4 batches x 2 cores),
tensor-parallel over heads within each pair (each core owns 4 of the 8 heads:
column-sharded W_q/W_k/W_v, row-sharded W_o).  Each core emits a partial
output projection for its batch; the host sums the two partials per batch.

Device-side layout choices:
  * All activations live transposed ([feature, seq]) so every matmul contracts
    over the partition dim with no on-chip transposes.
  * Scores are computed transposed (S_T[k, q] = K_h @ Q_h^T) so the valid-len
    key padding mask is a per-partition bias on the ACT exp instruction, and
    softmax needs no max-subtraction pass (scores are O(1) here; exp of the
    -1e6 masked entries underflows to exactly 0, matching the reference).
  * A ones-column interleaved into V makes the attnV matmul emit the softmax
    denominator for free (output row 64 of each head's [65, q] PSUM tile).
  * The key dim is truncated to max(valid_lens) rounded up to 128: dropped
    keys all have softmax weight exactly 0, so this is exact.
  * The whole matmul pipeline runs in bf16 with fp32 PSUM accumulation
    (plain fp32 matmuls are 4x slower on the PE and fp32 weight loads can't
    use fast-weight-load); softmax/normalization stay fp32.  Host converts
    inputs to bf16, which also halves the input DMA traffic.
"""

import ml_dtypes
import numpy as np

import concourse.bacc as bacc
import concourse.bass as bass
import concourse.mybir as mybir
from concourse import tile
from concourse.bass_utils import run_bass_kernel_spmd

F32 = mybir.dt.float32
F32R = mybir.dt.float32r
BF16 = mybir.dt.bfloat16

B, Q, KSEQ, D, H = 4, 2048, 2048, 512, 8
DH = D // H          # 64  head dim
HL = H // 2          # 4   local heads per core
DL = HL * DH         # 256 local features per core
NEG = -1.0e6
N_CORES = 8


def build_nc(KT: int):
    """Build the single-core SPMD program for a key length of KT (mult of 128)."""
    assert KT % 128 == 0 and 128 <= KT <= KSEQ
    KTC = KT // 128                      # number of 128-wide key chunks
    NQ = Q // 512                        # 4 q-chunks of 512
    KCH = [(s, min(512, KT - s)) for s in range(0, KT, 512)]
    EXP = mybir.ActivationFunctionType.Exp

    nc = bacc.Bacc("TRN2", target_bir_lowering=False, debug=False,
                   num_devices=N_CORES)

    def din(name, shape, dt=BF16):
        return nc.dram_tensor(name, shape, dt, kind="ExternalInput").ap()

    xq_d = din("xq_t", [D, Q])
    xk_d = din("xk_t", [D, KT])
    xv_d = din("xv_t", [D, KT])
    wq_d = din("wq_t", [D, DL])
    wk_d = din("wk_t", [D, DL])
    wv_d = din("wv_t", [D, DL])
    wo_d = din("wo_t", [DL, D])
    mask_d = din("mask", [128, KTC], F32)
    y_d = nc.dram_tensor("y_t", [D, Q], F32, kind="ExternalOutput").ap()

    with tile.TileContext(nc) as tc:
        with (
            # bf16 rounding on PSUM->SBUF copies is deliberate (see docstring)
            nc.allow_low_precision(reason="bf16 matmul operands"),
            tc.tile_pool(name="persist", bufs=1) as pp,
            tc.tile_pool(name="xpool", bufs=8) as xp,
            tc.tile_pool(name="cbuf", bufs=1) as cb,
            # 8 PSUM banks: psA 2x[128,512] (projections / broadcast / output
            # projection), psS 2x[128,1024] score tiles, psO 2x[65,512]
            # attention accumulators.
            tc.tile_pool(name="psA", bufs=2, space=bass.MemorySpace.PSUM) as psA,
            tc.tile_pool(name="psS", bufs=2, space=bass.MemorySpace.PSUM) as psS,
            tc.tile_pool(name="psO", bufs=1, space=bass.MemorySpace.PSUM) as psO,
        ):
            # ---- constants / weights / mask ----
            wq = [pp.tile([128, DL], BF16, tag=f"wq{i}", name=f"wq{i}") for i in range(4)]
            wk = [pp.tile([128, DL], BF16, tag=f"wk{i}", name=f"wk{i}") for i in range(4)]
            wv = [pp.tile([128, DL], BF16, tag=f"wv{i}", name=f"wv{i}") for i in range(4)]
            wo = [pp.tile([128, D], BF16, tag=f"wo{i}", name=f"wo{i}") for i in range(2)]
            for i in range(4):
                nc.sync.dma_start(wq[i][:], wq_d[i * 128:(i + 1) * 128, :])
            mask_sb = pp.tile([128, KTC], F32, tag="mask", name="mask_sb")
            nc.sync.dma_start(mask_sb[:], mask_d[:])
            onescr = pp.tile([128, DH], F32, tag="onescr", name="onescr")
            nc.vector.memset(onescr[:], 1.0)
            # row 64 is the broadcast-matmul lhsT (must share base partition
            # with the denominator row it multiplies against)
            ones_sb = pp.tile([65, DH], F32R, tag="ones", name="ones_sb")
            nc.vector.tensor_copy(ones_sb[64:65, :], onescr[64:65, :])

            # ---- Q projection:  q_t[o, q] = (Wq_loc/8) @ x_q  (transposed) ----
            # first input is DMA'd in 512-column chunks so the first matmul
            # group can start as early as possible
            xq = [xp.tile([128, Q], BF16, tag="x", name=f"x{i}") for i in range(4)]
            xk = [xp.tile([128, Q], BF16, tag="x", name=f"xk{i}") for i in range(4)]
            for i in range(4):
                nc.sync.dma_start(xq[i][:, 0:512],
                                  xq_d[i * 128:(i + 1) * 128, 0:512])
            # K-side loads issue ahead of the remaining q transfers so the
            # K projection isn't stuck ~19us deep in the sync DMA queue
            for i in range(4):
                nc.sync.dma_start(wk[i][:], wk_d[i * 128:(i + 1) * 128, :])
            for i in range(4):
                nc.sync.dma_start(xk[i][:, :KT], xk_d[i * 128:(i + 1) * 128, :])
            for i in range(4):
                nc.sync.dma_start(xq[i][:, 512:Q],
                                  xq_d[i * 128:(i + 1) * 128, 512:Q])
            for i in range(4):
                nc.sync.dma_start(wv[i][:], wv_d[i * 128:(i + 1) * 128, :])
            for i in range(2):
                nc.sync.dma_start(wo[i][:], wo_d[i * 128:(i + 1) * 128, :])
            q_t = [pp.tile([128, Q], BF16, tag=f"q_t{i}", name=f"q_t{i}") for i in range(2)]

            def qproj(ot, qs):
                ps = psA.tile([128, 512], F32, tag="proj", name="ps")
                for ic in range(4):
                    nc.tensor.matmul(
                        ps[:],
                        (wq[ic][:, ot * 128:(ot + 1) * 128]),
                        (xq[ic][:, qs * 512:(qs + 1) * 512]),
                        start=(ic == 0), stop=(ic == 3))
                nc.vector.tensor_copy(q_t[ot][:, qs * 512:(qs + 1) * 512], ps[:])

            for qs in range(NQ):
                qproj(0, qs)

            # ---- K projection:  k_t[o, k] ----
            k_t = [pp.tile([128, KT], BF16, tag=f"k_t{i}", name=f"k_t{i}") for i in range(2)]

            def kproj(ot, s, w):
                ps = psA.tile([128, 512], F32, tag="proj", name="ps")
                for ic in range(4):
                    nc.tensor.matmul(
                        ps[:, :w],
                        (wk[ic][:, ot * 128:(ot + 1) * 128]),
                        (xk[ic][:, s:s + w]),
                        start=(ic == 0), stop=(ic == 3))
                nc.vector.tensor_copy(k_t[ot][:, s:s + w], ps[:, :w])

            for (s, w) in KCH:
                kproj(0, s, w)
            for qs in range(NQ):
                qproj(1, qs)
            for (s, w) in KCH:
                kproj(1, s, w)

            # ---- V projection (emitted lazily, interleaved into the first
            # attention panel so the PE stream has no separate V phase and the
            # ACT engine starts on exps ~35us earlier).  Natural layout
            # v[k, o], heads interleaved with a ones column: per key-chunk
            # tile [128, 4*65], col h*65+64 == 1 (gives the softmax
            # denominator for free in the attnV matmul).
            xv = [xp.tile([128, Q], BF16, tag="x", name=f"x{i}") for i in range(4)]
            for i in range(4):
                nc.sync.dma_start(xv[i][:, :KT], xv_d[i * 128:(i + 1) * 128, :])
            v_sb = [pp.tile([128, HL * 65], BF16, tag=f"v{kt}", name=f"v{kt}") for kt in range(KTC)]

            def vproj(kt):
                ps = psA.tile([128, 512], F32, tag="proj", name="ps")
                for ic in range(4):
                    nc.tensor.matmul(
                        ps[:, :DL],
                        (xv[ic][:, kt * 128:(kt + 1) * 128]),
                        (wv[ic][:]),
                        start=(ic == 0), stop=(ic == 3))
                nc.vector.tensor_copy(v_sb[kt][:, 64::65], onescr[:, 0:HL])
                for h in range(HL):
                    nc.vector.tensor_copy(
                        v_sb[kt][:, h * 65:h * 65 + 64],
                        ps[:, h * 64:(h + 1) * 64])

            # ---- attention ----
            # q is processed in 1024-wide panels: two 512-wide scores matmuls
            # share one [128, 1024] PSUM tile so a single ACT exp covers both
            # (the mask bias is per-partition = per-key, constant across q).
            # The attnV matmuls run one k-chunk BEHIND the scores (software
            # pipeline) so the PE never stalls waiting for the exp that feeds
            # them -- a gap-free PE keeps the HAM clock at 2.4 GHz.
            o_pair = [pp.tile([128, Q], BF16, tag=f"oh{i}", name=f"o_pair{i}")
                      for i in range(2)]
            first_panel = True
            for qp in range(Q // 1024):
                q0 = qp * 1024
                for h in range(HL):
                    tl, po = h // 2, (h % 2) * 64
                    oA = psO.tile([65, 512], F32, tag="oA", name="oA")
                    oB = psO.tile([65, 512], F32, tag="oB", name="oB")

                    def attnv(p, kt, h=h, oA=oA, oB=oB):
                        for hf, o_ps in enumerate((oA, oB)):
                            nc.tensor.matmul(
                                o_ps[:],
                                (v_sb[kt][:, h * 65:h * 65 + 65]),
                                (p[:, hf * 512:(hf + 1) * 512]),
                                start=(kt == 0), stop=(kt == KTC - 1))

                    prev = None
                    for kt in range(KTC):
                        if first_panel:
                            vproj(kt)
                        s_ps = psS.tile([128, 1024], F32, tag="s", name="s_ps")
                        for hf in range(2):
                            nc.tensor.matmul(
                                s_ps[:, hf * 512:(hf + 1) * 512],
                                (k_t[tl][po:po + 64, kt * 128:(kt + 1) * 128]),
                                (q_t[tl][po:po + 64,
                                         q0 + hf * 512:q0 + (hf + 1) * 512]),
                                start=True, stop=True)
                        p_sb = cb.tile([128, 1024], BF16, tag="p", bufs=4,
                                       name="p_sb")
                        nc.scalar.activation(
                            p_sb[:], s_ps[:], EXP,
                            bias=mask_sb[:, kt:kt + 1], scale=1.0)
                        if prev is not None:
                            attnv(*prev)
                        prev = (p_sb, kt)
                    attnv(*prev)
                    first_panel = False
                    # normalize: o[dh, q] /= denom[q] (denom is o_ps row 64):
                    # stage denom in SBUF, broadcast over 64 partitions via a
                    # C=1 matmul, 64-lane fast reciprocal, then scale.
                    for hf, o_ps in enumerate((oA, oB)):
                        dn = cb.tile([65, 512], F32R, tag="dn", bufs=2,
                                     name="dn")
                        nc.vector.tensor_copy(dn[64:65, :], o_ps[64:65, :])
                        bc_ps = psA.tile([64, 512], F32, tag="proj",
                                         name="bc_ps")
                        nc.tensor.matmul(bc_ps[:], (ones_sb[64:65, :]),
                                         (dn[64:65, :]), start=True, stop=True)
                        inv_sb = cb.tile([64, 512], F32, tag="invb", bufs=2,
                                         name="inv_sb")
                        nc.vector.reciprocal_approx_fast(inv_sb[:], bc_ps[:])
                        cols = slice(q0 + hf * 512, q0 + (hf + 1) * 512)
                        if h % 2 == 0:
                            nc.vector.tensor_mul(
                                o_pair[tl][0:64, cols], o_ps[0:64, :],
                                inv_sb[:])
                        else:
                            # DVE lanes can't cross partitions; normalize into
                            # a scratch tile and DMA-hop it to partitions
                            # 64-127 so the output projection can run C=128
                            o_tmp = cb.tile([64, 512], BF16, tag="otmp",
                                            bufs=2, name="o_tmp")
                            nc.vector.tensor_mul(o_tmp[:], o_ps[0:64, :],
                                                 inv_sb[:])
                            nc.sync.dma_start(o_pair[tl][64:128, cols],
                                              o_tmp[:])

                # ---- output projection for this q-panel, on the psA slots
                # (idle during attention), overlapping the next panel ----
                for ot in range(4):
                    for qs in (2 * qp, 2 * qp + 1):
                        y_ps = psA.tile([128, 512], F32, tag="proj", name="ps")
                        for pr in range(2):
                            nc.tensor.matmul(
                                y_ps[:],
                                (wo[pr][:, ot * 128:(ot + 1) * 128]),
                                (o_pair[pr][:, qs * 512:(qs + 1) * 512]),
                                start=(pr == 0), stop=(pr == 1))
                        y_sb = cb.tile([128, 512], F32, tag="y", bufs=2,
                                       name="y_sb")
                        nc.vector.tensor_copy(y_sb[:], y_ps[:])
                        nc.sync.dma_start(
                            y_d[ot * 128:(ot + 1) * 128,
                                qs * 512:(qs + 1) * 512],
                            y_sb[:])

    nc.compile()
    return nc


def make_in_maps(queries, keys, values, valid_lens, W_q, W_k, W_v, W_o, KT):
    queries = np.asarray(queries, np.float32)
    keys = np.asarray(keys, np.float32)
    values = np.asarray(values, np.float32)
    W_q = np.asarray(W_q, np.float32)
    W_k = np.asarray(W_k, np.float32)
    W_v = np.asarray(W_v, np.float32)
    W_o = np.asarray(W_o, np.float32)
    vl = np.asarray(valid_lens).astype(np.int64)
    in_maps = []
    for c in range(N_CORES):
        b, hg = c // 2, c % 2
        sl = slice(hg * DL, (hg + 1) * DL)
        m = np.where(np.arange(KT) < vl[b], 0.0, NEG).astype(np.float32)
        bf = ml_dtypes.bfloat16
        in_maps.append({
            "xq_t": np.ascontiguousarray(queries[b].T).astype(bf),
            "xk_t": np.ascontiguousarray(keys[b, :KT].T).astype(bf),
            "xv_t": np.ascontiguousarray(values[b, :KT].T).astype(bf),
            "wq_t": np.ascontiguousarray((W_q[sl, :] / 8.0).T).astype(bf),
            "wk_t": np.ascontiguousarray(W_k[sl, :].T).astype(bf),
            "wv_t": np.ascontiguousarray(W_v[sl, :].T).astype(bf),
            "wo_t": np.ascontiguousarray(W_o[:, sl].T).astype(bf),
            "mask": np.ascontiguousarray(m.reshape(KT // 128, 128).T),
        })
    return in_maps


def pick_kt(valid_lens):
    vl_max = int(np.asarray(valid_lens).max())
    return int(min(KSEQ, max(128, ((vl_max + 127) // 128) * 128)))


def kernel(queries, keys, values, valid_lens, W_q, W_k, W_v, W_o):
    KT = pick_kt(valid_lens)
    nc = build_nc(KT)
    in_maps = make_in_maps(queries, keys, values, valid_lens,
                           W_q, W_k, W_v, W_o, KT)
    res = run_bass_kernel_spmd(nc, in_maps, list(range(N_CORES))).results
    out = np.empty((B, Q, D), np.float32)
    for b in range(B):
        out[b] = (res[2 * b]["y_t"] + res[2 * b + 1]["y_t"]).T
    return out



# revision 2
# speedup vs baseline: 1.0408x; 1.0408x over previous
"""Multi-head attention (B=4, Q=K=2048, D=512, H=8) on 8 TRN2 NeuronCores.

Sharding: head-parallel with per-batch key truncation.  Core c owns head c
of ALL four batches ("slots").  Each slot's key range is truncated to its
own batch's valid_len (rounded up to 128), so per-core attention work is
sum_b ceil(vl_b/128) chunks instead of 4*max_b chunks -- load-balanced for
any valid_lens distribution, and never worse than batch-parallel.

Per slot the core computes q/k/v projections for its single head, scores,
masked exp (no max pass; exp of -1e6 masked entries underflows to 0),
attn@V with an interleaved ones-column that yields the softmax denominator
for free, normalization via a bf16 C=1 broadcast matmul + fast reciprocal,
and a C=64 output projection against its head's W_o columns, producing a
partial y per batch.  The host sums the 8 cores' partials.  No collectives.

Schedule notes (every engine queue is strict FIFO -- emission order IS the
schedule, and a DMA_DIRECT2D costs ~0.6us of issuing-engine time):
  * all inputs are HOST-PACKED so each tensor is one or two big 2D DMAs
    (~20 DMA instructions total instead of ~100 -- issue serialization on
    the sync engine was costing 60us).
  * slot0's x tensors arrive in two key/q-aligned halves; its first scores
    run ~8us in while the rest streams.
  * slot s+1's q/k projections and each panel's output projection are
    "filler" units popped one per chunk inside the attention streams, so
    the PE absorbs ACT's exp pacing (1147ns/chunk vs PE ~900ns) without
    idling (idle PE re-throttles the HAM clock gate to 1.2 GHz).
  * each panel's normalize is deferred into the next panel's chunk stream;
    v projections run two chunks ahead of their attnV use.
  * y PSUM->SBUF bf16 drains alternate between DVE and ACT (fp32 PSUM
    reads are the scarce shared resource); y DMAs ride the sync queue.
  * 16 dummy matmuls at t=0 warm the PE; a dummy exp preloads ACT tables.
"""

import ml_dtypes
import numpy as np

import concourse.bacc as bacc
import concourse.bass as bass
import concourse.mybir as mybir
from concourse import tile
from concourse.bass_utils import run_bass_kernel_spmd

F32 = mybir.dt.float32
BF16 = mybir.dt.bfloat16

B, Q, KSEQ, D, H = 4, 2048, 2048, 512, 8
DH = D // H          # 64 head dim
NEG = -1.0e6
N_CORES = 8


def build_nc(ktcs):
    """Single-core SPMD program; ktcs = per-slot key-chunk counts (desc)."""
    assert len(ktcs) == B and all(1 <= k <= KSEQ // 128 for k in ktcs)
    NKS = [k * 128 for k in ktcs]
    MOFF = [sum(ktcs[:s]) for s in range(B)]      # mask column offsets
    KH0 = (ktcs[0] + 1) // 2                      # slot0 half sizes (chunks)
    NH0 = KH0 * 128
    L0 = [NH0, NKS[0] - NH0]                      # keys per half
    EXP = mybir.ActivationFunctionType.Exp

    nc = bacc.Bacc("TRN2", target_bir_lowering=False, debug=False,
                   num_devices=N_CORES)

    def din(name, shape, dt=BF16):
        return nc.dram_tensor(name, shape, dt, kind="ExternalInput").ap()

    # host-packed layouts (see make_in_maps): feature-chunk-major columns
    xq_d = [din(f"xq{s}", [128, 4 * Q]) for s in range(B)]
    xk0_d = din("xk0", [128, 4 * NKS[0]])
    xv0_d = din("xv0", [128, 4 * NKS[0]])
    xkv_d = [None] + [din(f"xkv{s}", [128, 8 * NKS[s]]) for s in range(1, B)]
    wqkv_d = din("wqkv", [128, 3 * 4 * DH])
    wo_d = din("wo_t", [DH, D])
    mask_d = din("maskall", [128, sum(ktcs)], F32)
    y_d = [nc.dram_tensor(f"y{s}", [128, 8 * Q // 2], BF16,
                          kind="ExternalOutput").ap() for s in range(B)]

    with tile.TileContext(nc) as tc:
        with (
            nc.allow_low_precision(reason="bf16 matmul operands"),
            tc.tile_pool(name="persist", bufs=1) as pp,
            tc.tile_pool(name="cbuf", bufs=1) as cb,
            tc.tile_pool(name="psA", bufs=2, space=bass.MemorySpace.PSUM) as psA,
            tc.tile_pool(name="psS", bufs=2, space=bass.MemorySpace.PSUM) as psS,
            tc.tile_pool(name="psO", bufs=1, space=bass.MemorySpace.PSUM) as psO,
        ):
            # ---- PE warm-up + ACT exp-table preload (both off critical path)
            warm = pp.tile([128, 512], BF16, tag="warm", name="warm")
            nc.vector.memset(warm[:], 0.0)
            for i in range(9):
                wps = psA.tile([128, 512], F32, tag="proj", name="wps")
                nc.tensor.matmul(wps[:], (warm[:, 0:128]), (warm[:]),
                                 start=True, stop=True)
            pwarm = cb.tile([128, 64], BF16, tag="pwarm", name="pwarm")
            nc.scalar.activation(pwarm[:], warm[:, 0:64], EXP, scale=1.0)

            # ---- constants ----
            onescr = pp.tile([128, DH], F32, tag="onescr", name="onescr")
            nc.vector.memset(onescr[:], 1.0)
            ones_sb = pp.tile([65, DH], BF16, tag="ones", name="ones_sb")
            nc.vector.tensor_copy(ones_sb[64:65, :], onescr[64:65, :])

            # ---- input DMAs (sync queue, ordered by first use) ----
            wqkv = pp.tile([128, 12 * DH], BF16, tag="wqkv", name="wqkv")
            wo = pp.tile([DH, D], BF16, tag="wo", name="wo")
            mask_sb = pp.tile([128, sum(ktcs)], F32, tag="mask", name="mask")
            nc.sync.dma_start(wqkv[:], wqkv_d[:])
            nc.sync.dma_start(mask_sb[:], mask_d[:])
            wq = wqkv[:, 0:4 * DH]
            wk = wqkv[:, 4 * DH:8 * DH]
            wv = wqkv[:, 8 * DH:12 * DH]

            xqt = [pp.tile([128, 4 * Q], BF16, tag=f"xq{s}", name=f"xq{s}")
                   for s in range(B)]
            xk0t = pp.tile([128, 4 * NKS[0]], BF16, tag="xk0", name="xk0")
            xv0t = pp.tile([128, 4 * NKS[0]], BF16, tag="xv0", name="xv0")
            xkvt = [None] + [pp.tile([128, 8 * NKS[s]], BF16, tag=f"xkv{s}",
                                     name=f"xkv{s}") for s in range(1, B)]
            # slot0's first halves ride the sync ring (first scores ~11us in);
            # the bulk rides the ACT-engine ring CONCURRENTLY (issued before
            # any exp, dependency-free) so late slots never starve -- all
            # transfers of one issuing engine serialize on one hardware ring
            nc.sync.dma_start(xqt[0][:, 0:4096], xq_d[0][:, 0:4096])
            nc.sync.dma_start(xk0t[:, 0:4 * L0[0]], xk0_d[:, 0:4 * L0[0]])
            nc.sync.dma_start(xv0t[:, 0:4 * L0[0]], xv0_d[:, 0:4 * L0[0]])
            nc.sync.dma_start(xqt[0][:, 4096:8192], xq_d[0][:, 4096:8192])
            if L0[1]:
                nc.sync.dma_start(xk0t[:, 4 * L0[0]:], xk0_d[:, 4 * L0[0]:])
                nc.sync.dma_start(xv0t[:, 4 * L0[0]:], xv0_d[:, 4 * L0[0]:])
            nc.sync.dma_start(wo[:], wo_d[:])
            for s in range(1, B):
                nc.sync.dma_start(xqt[s][:], xq_d[s][:])
                nc.sync.dma_start(xkvt[s][:], xkv_d[s][:])

            # column address of q position / key position per feature-chunk i
            def xq_ap(s, i, q0, w):
                if s == 0:
                    h = q0 // 1024
                    return xqt[0][:, h * 4096 + i * 1024 + (q0 - h * 1024):
                                  h * 4096 + i * 1024 + (q0 - h * 1024) + w]
                return xqt[s][:, i * Q + q0:i * Q + q0 + w]

            def xk_ap(s, i, c0, w):
                if s == 0:
                    h = 0 if c0 < NH0 else 1
                    base = h * 4 * L0[0] + i * L0[h] + (c0 - h * NH0)
                    return xk0t[:, base:base + w]
                nk = NKS[s]
                return xkvt[s][:, i * 2 * nk + c0:i * 2 * nk + c0 + w]

            def xv_ap(s, i, c0, w):
                if s == 0:
                    h = 0 if c0 < NH0 else 1
                    base = h * 4 * L0[0] + i * L0[h] + (c0 - h * NH0)
                    return xv0t[:, base:base + w]
                nk = NKS[s]
                return xkvt[s][:, i * 2 * nk + nk + c0:i * 2 * nk + nk + c0 + w]

            def kblocks(s):
                """512-wide key blocks, aligned to slot0's half boundary."""
                edges = sorted({0, NKS[s]} | ({NH0} if s == 0 else set()))
                out = []
                for a, b in zip(edges, edges[1:]):
                    for c0 in range(a, b, 512):
                        out.append((c0, min(512, b - c0)))
                return out

            # ---- projections ----
            q_t = [pp.tile([DH, Q], BF16, tag=f"q_t{s}", name=f"q_t{s}")
                   for s in range(B)]
            k_t = [pp.tile([DH, NKS[s]], BF16, tag=f"k_t{s}", name=f"k_t{s}")
                   for s in range(B)]
            v_sb = [pp.tile([128, ktcs[s] * 65], BF16, tag=f"v{s}",
                            name=f"v{s}") for s in range(B)]

            def qproj(s, qs):
                ps = psA.tile([128, 512], F32, tag="proj", name="ps")
                for ic in range(4):
                    nc.tensor.matmul(
                        ps[0:DH, :],
                        (wq[:, ic * DH:(ic + 1) * DH]),
                        (xq_ap(s, ic, qs * 512, 512)),
                        start=(ic == 0), stop=(ic == 3))
                nc.vector.tensor_copy(q_t[s][:, qs * 512:(qs + 1) * 512],
                                      ps[0:DH, :])

            def kproj(s, b0, w):
                ps = psA.tile([128, 512], F32, tag="proj", name="ps")
                for ic in range(4):
                    nc.tensor.matmul(
                        ps[0:DH, :w],
                        (wk[:, ic * DH:(ic + 1) * DH]),
                        (xk_ap(s, ic, b0, w)),
                        start=(ic == 0), stop=(ic == 3))
                nc.vector.tensor_copy(k_t[s][:, b0:b0 + w], ps[0:DH, :w])

            def vproj(s, kt):
                ps = psA.tile([128, 512], F32, tag="proj", name="ps")
                for ic in range(4):
                    nc.tensor.matmul(
                        ps[:, 0:DH],
                        (xv_ap(s, ic, kt * 128, 128)),
                        (wv[:, ic * DH:(ic + 1) * DH]),
                        start=(ic == 0), stop=(ic == 3))
                nc.vector.tensor_copy(v_sb[s][:, kt * 65:kt * 65 + 64],
                                      ps[:, 0:DH])

            def proj_doses(s, qs_from=0, kb_from=0):
                ds = [(lambda qs=qs, s=s: qproj(s, qs))
                      for qs in range(qs_from, 4)]
                ds += [(lambda b0=b0, w=w, s=s: kproj(s, b0, w))
                       for (b0, w) in kblocks(s)[kb_from:]]
                return ds

            # upfront: only what the first scores chunk needs
            qproj(0, 0)
            qproj(0, 1)
            nb0 = len([b for (b, w) in kblocks(0) if b < NH0])
            for (b0, w) in kblocks(0)[:nb0]:
                kproj(0, b0, w)

            # ---- attention ----
            o_sb = [pp.tile([DH, Q], BF16, tag=f"o{s}", name=f"o{s}")
                    for s in range(B)]
            pending = [None]   # normalize closure for the previous panel
            dose_fifo = []     # projections with a hard deadline
            out_fifo = []      # output-projection units (soft deadline)
            ucount = [0]

            def outproj_unit(s, q0, ot, yst):
                def unit():
                    for qh in range(2):
                        y_ps = psA.tile([128, 512], F32, tag="proj",
                                        name="y_ps")
                        nc.tensor.matmul(
                            y_ps[:],
                            (wo[:, ot * 128:(ot + 1) * 128]),
                            (o_sb[s][:, q0 + qh * 512:q0 + (qh + 1) * 512]),
                            start=True, stop=True)
                        ucount[0] += 1
                        dst = yst[:, ot * 1024 + qh * 512:
                                  ot * 1024 + (qh + 1) * 512]
                        if ucount[0] % 2 == 0:
                            nc.scalar.copy(dst, y_ps[:])
                        else:
                            nc.vector.tensor_copy(dst, y_ps[:])
                    if ot == 3:
                        panel = q0 // 1024
                        nc.sync.dma_start(
                            y_d[s][:, panel * 4096:(panel + 1) * 4096],
                            yst[:])
                return unit

            def make_finish(s, q0, oA, oB):
                def fin():
                    # normalize: o[dh, q] /= denom[q] (row 64 of oA/oB);
                    # bf16 denominator row -> bf16 broadcast matmul
                    for hf, o_ps in enumerate((oA, oB)):
                        dn = cb.tile([65, 512], BF16, tag="dn", bufs=2,
                                     name="dn")
                        nc.vector.tensor_copy(dn[64:65, :], o_ps[64:65, :])
                        bc_ps = psA.tile([64, 512], F32, tag="proj",
                                         name="bc_ps")
                        nc.tensor.matmul(bc_ps[:], (ones_sb[64:65, :]),
                                         (dn[64:65, :]), start=True, stop=True)
                        inv_sb = cb.tile([64, 512], F32, tag="invb", bufs=2,
                                         name="inv_sb")
                        nc.vector.reciprocal_approx_fast(inv_sb[:], bc_ps[:])
                        cols = slice(q0 + hf * 512, q0 + (hf + 1) * 512)
                        nc.vector.tensor_mul(o_sb[s][:, cols],
                                             o_ps[0:DH, :], inv_sb[:])
                    yst = cb.tile([128, 4096], BF16, tag="yst", bufs=2,
                                  name="yst")
                    out_fifo.extend(outproj_unit(s, q0, ot, yst)
                                    for ot in range(4))
                return fin

            for s in range(B):
                KTC = ktcs[s]
                # this slot's projections MUST be emitted before its scores
                for f in dose_fifo:
                    f()
                dose_fifo.clear()
                if s == 0:
                    # slot0's own second-half projections dose into panel0
                    dose_fifo.extend(proj_doses(0, qs_from=2, kb_from=nb0))
                # v ones-columns, once per slot (gives softmax denominator)
                nc.vector.tensor_copy(v_sb[s][:, 64::65], onescr[:, 0:KTC])
                for panel in range(2):
                    if panel == 1:
                        for f in dose_fifo:   # slot0 leftovers: p1 needs them
                            f()
                        dose_fifo.clear()
                        if s + 1 < B:
                            dose_fifo.extend(proj_doses(s + 1))
                    q0 = panel * 1024
                    oA = psO.tile([65, 512], F32, tag="oA", name="oA")
                    oB = psO.tile([65, 512], F32, tag="oB", name="oB")

                    def attnv(p, kt, s=s, oA=oA, oB=oB, KTC=KTC):
                        for hf, o_ps in enumerate((oA, oB)):
                            nc.tensor.matmul(
                                o_ps[:],
                                (v_sb[s][:, kt * 65:kt * 65 + 65]),
                                (p[:, hf * 512:(hf + 1) * 512]),
                                start=(kt == 0), stop=(kt == KTC - 1))

                    # v projections run 2 chunks ahead of their attnv use
                    if panel == 0:
                        vproj(s, 0)
                        if KTC > 1:
                            vproj(s, 1)
                    prev = None
                    for kt in range(KTC):
                        if panel == 0 and kt + 2 < KTC:
                            vproj(s, kt + 2)
                        s_ps = psS.tile([128, 1024], F32, tag="s", name="s_ps")
                        for hf in range(2):
                            nc.tensor.matmul(
                                s_ps[:, hf * 512:(hf + 1) * 512],
                                (k_t[s][:, kt * 128:(kt + 1) * 128]),
                                (q_t[s][:, q0 + hf * 512:q0 + (hf + 1) * 512]),
                                start=True, stop=True)
                        p_sb = cb.tile([128, 1024], BF16, tag="p", bufs=4,
                                       name="p_sb")
                        nc.scalar.activation(
                            p_sb[:], s_ps[:], EXP,
                            bias=mask_sb[:, MOFF[s] + kt:MOFF[s] + kt + 1],
                            scale=1.0)
                        if kt == 0 and pending[0] is not None:
                            pending[0]()
                            pending[0] = None
                        if prev is not None:
                            attnv(*prev)
                        if dose_fifo and (panel == 1 or kt >= 4):
                            dose_fifo.pop(0)()
                        elif out_fifo and (panel == 1 or len(out_fifo) > 2):
                            out_fifo.pop(0)()
                        prev = (p_sb, kt)
                    attnv(*prev)
                    pending[0] = make_finish(s, q0, oA, oB)
            pending[0]()
            for f in out_fifo:
                f()

    nc.compile()
    return nc


def plan(valid_lens):
    """Slot order (batches sorted by descending chunk count) + chunk counts."""
    vl = np.asarray(valid_lens).astype(np.int64)
    ktc = [max(1, int((int(v) + 127) // 128)) for v in vl]
    order = sorted(range(B), key=lambda b: -ktc[b])
    return order, tuple(ktc[b] for b in order)


def make_in_maps(queries, keys, values, valid_lens, W_q, W_k, W_v, W_o,
                 order, ktcs):
    bf = ml_dtypes.bfloat16
    queries = np.asarray(queries, np.float32)
    keys = np.asarray(keys, np.float32)
    values = np.asarray(values, np.float32)
    W_q = np.asarray(W_q, np.float32)
    W_k = np.asarray(W_k, np.float32)
    W_v = np.asarray(W_v, np.float32)
    W_o = np.asarray(W_o, np.float32)
    vl = np.asarray(valid_lens).astype(np.int64)
    KH0 = (ktcs[0] + 1) // 2
    NH0 = KH0 * 128

    def packw(w):  # [64, 512] head slice -> lhsT chunks packed [128, 256]
        wt = np.ascontiguousarray(w.T)          # [512, 64]
        return np.concatenate([wt[i * 128:(i + 1) * 128, :] for i in range(4)],
                              axis=1).astype(bf)

    def chunkcat(a, col_ranges):  # a: [512, N] -> [128, 4*sum(w)] packed
        parts = []
        for (c0, c1) in col_ranges:
            parts.append(np.concatenate(
                [a[i * 128:(i + 1) * 128, c0:c1] for i in range(4)], axis=1))
        return np.ascontiguousarray(np.concatenate(parts, axis=1))

    common = {}
    masks = []
    for s, b in enumerate(order):
        nk = ktcs[s] * 128
        qT = queries[b].T.astype(bf)
        kT = keys[b, :nk].T.astype(bf)
        vT = values[b, :nk].T.astype(bf)
        if s == 0:
            common["xq0"] = chunkcat(qT, [(0, 1024), (1024, 2048)])
            common["xk0"] = chunkcat(kT, [(0, NH0), (NH0, nk)] if nk > NH0
                                     else [(0, NH0)])
            common["xv0"] = chunkcat(vT, [(0, NH0), (NH0, nk)] if nk > NH0
                                     else [(0, NH0)])
        else:
            common[f"xq{s}"] = chunkcat(qT, [(0, 2048)])
            kv = np.concatenate([
                np.concatenate([kT[i * 128:(i + 1) * 128, :],
                                vT[i * 128:(i + 1) * 128, :]], axis=1)
                for i in range(4)], axis=1)
            common[f"xkv{s}"] = np.ascontiguousarray(kv)
        m = np.where(np.arange(nk) < vl[b], 0.0, NEG).astype(np.float32)
        masks.append(m.reshape(ktcs[s], 128).T)
    common["maskall"] = np.ascontiguousarray(np.concatenate(masks, axis=1))

    in_maps = []
    for c in range(N_CORES):
        sl = slice(c * DH, (c + 1) * DH)
        im = dict(common)
        im["wqkv"] = np.ascontiguousarray(np.concatenate(
            [packw(W_q[sl, :] / 8.0), packw(W_k[sl, :]), packw(W_v[sl, :])],
            axis=1))
        im["wo_t"] = np.ascontiguousarray(W_o[:, sl].T).astype(bf)
        in_maps.append(im)
    return in_maps


def unpack_y(arr):
    """[128, 8192] device layout -> [512, 2048] partial y."""
    y = np.empty((D, Q), np.float32)
    a = np.asarray(arr, dtype=np.float32)
    for panel in range(2):
        for ot in range(4):
            y[ot * 128:(ot + 1) * 128, panel * 1024:(panel + 1) * 1024] = \
                a[:, panel * 4096 + ot * 1024:panel * 4096 + (ot + 1) * 1024]
    return y


def kernel(queries, keys, values, valid_lens, W_q, W_k, W_v, W_o):
    order, ktcs = plan(valid_lens)
    nc = build_nc(ktcs)
    in_maps = make_in_maps(queries, keys, values, valid_lens,
                           W_q, W_k, W_v, W_o, order, ktcs)
    res = run_bass_kernel_spmd(nc, in_maps, list(range(N_CORES))).results
    out = np.zeros((B, Q, D), np.float32)
    for s, b in enumerate(order):
        acc = np.zeros((D, Q), np.float32)
        for c in range(N_CORES):
            acc += unpack_y(res[c][f"y{s}"])
        out[b] = acc.T
    return out
